# revision 49
# baseline (speedup 1.0000x reference)
"""ClassAttention kernel for 8x TRN2 NeuronCores — fp8 DoubleRow rewrite.

Reference computation (per batch element):
    qkv = x @ qkv_w.T + qkv_b                      # [N, 3C]
    q, k, v = split(qkv)                           # heads H=12, D=64
    s = softmax((q_cls . k) / sqrt(D))             # class-token query only
    cls = (s @ v) @ proj_w.T + proj_b              # [1, C]
    out = concat([cls, x[1:]])                     # rows 1..N pass through

Only the class token row changes, so the device computes just the [B, C]
cls output (shipped transposed as clsT in a descriptor-friendly
[128, 6, B] layout); rows 1..N pass through on the host.  Data-parallel
over batch: 8 batches per core, no collectives.

Algebraic structure (inherited from the bf16 baseline):
  - k-projection folds into x-space:  s[b,h,n] = sum_c Wt[c,bh] x[b,n,c]
    with Wt = wk.T @ blockdiag(q + qb) computed once on device; no k
    vector is materialized.  k-bias cancels in softmax; q-bias is added
    during the Qblk blockdiag scatter (per-partition scalar add).
  - v-projection commutes with the attention average: the kernel averages
    x (ZT = x.T @ p) and projects through wv once; v-bias folds into the
    proj bias on the host.
  - softmax skips the max-shift; exp(s - 1) keeps the fp8 range safe and
    the constant cancels in the 1/sum, which is applied per (b,h) column
    during the ZT psum evacuation.

What is new vs the 82.2us bf16 baseline (2.5x):
  - everything is fp8(e4m3) except psum (f32), the exp sums/rden (f32)
    and the cls output (bf16, widened to f32 on the host).  Error is dominated by the score-path
    quantization; measured full-output rel err 9.9e-3 vs the 2e-2 gate
    on the fixed-seed reference.  DoubleRow fp8 matmuls (two 128-row
    k-tiles per instruction, 0.5 cycles/row) carry all heavy
    contractions; fp8 operand tiles are padded so every DoubleRow k-pair
    slice has a 64-multiple byte stride (walrus ISA requirement), and
    DoubleRow outputs always sit at psum partition 0 (dst partition 64
    fails the s3d3 ISA check; those matmuls run non-DR).
  - every stage computes the TRANSPOSED output with a small moving free
    dim (qT, sT directly in [n, (b h)] form, ZT, oT head-diagonal blocks
    directly, clsT), so the kernel contains ZERO PE transposes of data
    and psum evacuations are few and wide.
  - 30 exact-size DMAs instead of 67 (HWDGE issue is ~630ns each and
    gated the baseline).  Modeled DMA stream is gapless at 360 GB/s:
    wq,wk2,xcls,qbd,wv | xT per batch | x2 per batch (512-row body +
    65-row tail) | wp, pbT last — so the only post-stream work is
    proj (18 DR matmuls) -> bias add -> one output DMA, and the last
    x2 batch's Z/oT chain hides under the wp transfer.
  - small tensors (xcls, qbd, pbT, clsT) use partition-major [128,6,k]
    DRAM layouts: 128 descriptors instead of 768 (descriptor-floor
    bound at 7ns/desc).

Per-core stages (b = 0..8 batches, c in 6 chunks of 128):
  qT[o, b]       36 matmuls          Qblk[o, (b h)]  12 scatter+qb adds
  Wt[c, (b h)]   18 DR + 6 copies    sT[n, (b h)]    120 DR matmuls
  pT = exp(sT-1) 2 Act ops -> fp8    sums/rden       5 ones-matmuls + recip
  rdenB[o,(b h)] 1 outer-product     ZT[c, b]        144 DR + 8 TT-mul evacs
  oT[o', b]      216 small matmuls (groups 4/3/1, tracks x2 arrivals)
  clsT[j, b]     18 DR + pbT add -> one output DMA

Modeled 32643 ns; measured full-output rel err 1.03e-2 (gate 2e-2).
"""

import functools

import numpy as np
import ml_dtypes

import concourse.bass as bass
import concourse.tile as tile
from concourse import bacc, mybir
from concourse import bass_utils

BF16 = mybir.dt.bfloat16
F8 = mybir.dt.float8e4
F32 = mybir.dt.float32
NPBF16 = ml_dtypes.bfloat16
NPF8 = ml_dtypes.float8_e4m3
DR = mybir.MatmulPerfMode.DoubleRow

B, N, C = 64, 577, 768
H, D = 12, 64
NCORES = 8
BPC = B // NCORES          # 8 batches per core
CT = C // 128              # 6 chunks of the feature dim
NT = 5                     # token tiles of 128 (last holds 65)
NTAIL = N - 4 * 128        # 65
SCALE = D ** -0.5          # folded into wq on the host


def build_module():
    nc = bacc.Bacc("TRN2", target_bir_lowering=False, debug=False)

    xT_d = nc.dram_tensor("xT", [C, BPC, N], F8, kind="ExternalInput")
    x2_d = nc.dram_tensor("x2", [BPC * N, C], F8, kind="ExternalInput")
    wq_d = nc.dram_tensor("wq", [C, C], F8, kind="ExternalInput")      # [c, o]
    wk2_d = nc.dram_tensor("wk2", [C, C], F8, kind="ExternalInput")    # [o, c]
    wv_d = nc.dram_tensor("wv", [C, C], F8, kind="ExternalInput")      # [c, o]
    wp_d = nc.dram_tensor("wp", [C, C], F8, kind="ExternalInput")      # [c, o]
    xcls_d = nc.dram_tensor("xcls", [128, CT, BPC], F8, kind="ExternalInput")
    qbd_d = nc.dram_tensor("qbd", [128, CT], F32, kind="ExternalInput")
    pbT_d = nc.dram_tensor("pbT", [128, CT, BPC], F32, kind="ExternalInput")
    clsT_d = nc.dram_tensor("clsT", [128, CT, BPC], BF16, kind="ExternalOutput")

    AF = mybir.ActivationFunctionType

    with tile.TileContext(nc) as tc:
        with (
            tc.tile_pool(name="sb", bufs=1) as sb,
            tc.tile_pool(name="psA", bufs=2, space="PSUM") as psA,
            tc.tile_pool(name="psW", bufs=1, space="PSUM") as psW,
            tc.tile_pool(name="psS", bufs=1, space="PSUM") as psS,
            tc.tile_pool(name="psR", bufs=1, space="PSUM") as psR,
            tc.tile_pool(name="psZ", bufs=3, space="PSUM") as psZ,
        ):
            # ---- DMAs, in consumption order (one channel, serialized) ----
            wq = sb.tile([128, CT, C], F8, tag="wq")
            nc.sync.dma_start(
                wq[:], wq_d.ap().rearrange("(a p) o -> p a o", p=128))
            wk2 = sb.tile([128, CT, C], F8, tag="wk2")
            nc.sync.dma_start(
                wk2[:], wk2_d.ap().rearrange("(a p) o -> p a o", p=128))
            xcls = sb.tile([128, CT, BPC], F8, tag="xcls")
            nc.sync.dma_start(xcls[:], xcls_d.ap())
            qbd = sb.tile([128, CT, 1], F32, tag="qbd")
            nc.sync.dma_start(qbd[:, :, 0], qbd_d.ap())
            # x in c-major layout, one DMA per batch; rows padded to 640 so
            # DoubleRow k-tile-pair slices have a 64-multiple stride (walrus
            # ISA requirement on Ldweights)
            wv = sb.tile([128, CT, C], F8, tag="wv")
            nc.sync.dma_start(
                wv[:], wv_d.ap().rearrange("(a p) o -> p a o", p=128))
            wp = sb.tile([128, CT, C], F8, tag="wp")
            wpr = wp_d.ap().rearrange("(a p) o -> p a o", p=128)
            nc.sync.dma_start(wp[:, 0:4, :], wpr[:, 0:4, :])
            xTs = []
            for b in range(BPC):
                xt = sb.tile([128, CT, 640], F8, tag=f"xT{b}")
                nc.sync.dma_start(
                    xt[:, :, 0:N],
                    xT_d.ap()[:, b, :].rearrange("(a p) t -> p a t", p=128))
                xTs.append(xt)
            # x in token-major layout, two exact-size DMAs per batch (the
            # 512-row body, then the 65-row tail)
            x2s = []
            x2ts = []
            for b in range(BPC):
                x2 = sb.tile([128, 4, C], F8, tag=f"x2{b}")
                nc.sync.dma_start(
                    x2[:],
                    x2_d.ap()[b * N:b * N + 512, :]
                    .rearrange("(a p) c -> p a c", p=128))
                x2t = sb.tile([NTAIL, C], F8, tag=f"x2t{b}")
                nc.sync.dma_start(
                    x2t[:], x2_d.ap()[b * N + 512:b * N + N, :])
                x2s.append(x2)
                x2ts.append(x2t)
            # wp's last third is the LAST weight input: only one matmul
            # per proj group (k-rotated to the end) waits on it
            nc.sync.dma_start(wp[:, 4:6, :], wpr[:, 4:6, :])
            # pbT is the very last input: the only work behind it is the
            # final bias add
            pbT = sb.tile([128, CT, BPC], F32, tag="pbT")
            nc.sync.dma_start(pbT[:], pbT_d.ap())

            # ---- small constants ----
            ones8 = sb.tile([128, 2, 64], F8, tag="ones8")
            nc.vector.memset(ones8[:], 1.0)
            negone = sb.tile([128, 1], F32, tag="negone")
            nc.vector.memset(negone[:], -1.0)
            onesf = sb.tile([1, 128], F32, tag="onesf")
            nc.vector.memset(onesf[:], 1.0)
            Qblk = sb.tile([128, CT, 128], F8, tag="Qblk")
            nc.vector.memset(Qblk[:], 0.0)

            # fp8 operand tiles are padded so every DoubleRow k-pair slice
            # has a 64-multiple stride
            Wt = sb.tile([128, CT, 128], F8, tag="Wt")
            pT = sb.tile([128, NT, BPC, 16], F8, tag="pT")
            rden = sb.tile([1, BPC * H], F32, tag="rden")
            rdenB = sb.tile([128, BPC, H], F32, tag="rdenB")
            ZT = sb.tile([128, CT, BPC, 16], F8, tag="ZT")
            oT = sb.tile([128, CT, 64], F8, tag="oT")
            clsT_sb = sb.tile([128, CT, BPC], BF16, tag="clsT_sb")

            # ---- qT[o, b]: 36 bf16 matmuls, out free dim 8 ----
            pq = psA.tile([128, CT, BPC], F32, tag="A")
            for oc in range(CT):
                for ck in range(CT):
                    nc.tensor.matmul(
                        pq[:, oc, :],
                        wq[:, ck, 128 * oc:128 * (oc + 1)],
                        xcls[:, ck, :],
                        start=(ck == 0), stop=(ck == CT - 1))

            # ---- Qblk[o, (b h)]: blockdiag scatter of qT + q-bias
            #      (per-partition scalar add), cast fp8 ----
            QblkV = Qblk[:, :, 0:BPC * H].rearrange(
                "p a (b h) -> p a b h", h=H)
            for oc in range(CT):
                for j in range(2):
                    h = 2 * oc + j
                    nc.vector.tensor_scalar_add(
                        QblkV[64 * j:64 * (j + 1), oc, :, h],
                        pq[64 * j:64 * (j + 1), oc, :],
                        qbd[64 * j:64 * (j + 1), oc, :])

            # ---- Wt[c, (b h)] = wk2.T @ Qblk + wtqb, cast fp8 ----
            for cj in range(CT):
                pw = psW.tile([128, BPC * H], F32, tag="W")
                for t in range(3):
                    nc.tensor.matmul(
                        pw[:], wk2[:, 2 * t:2 * t + 2, 128 * cj:128 * (cj + 1)],
                        Qblk[:, 2 * t:2 * t + 2, 0:BPC * H],
                        start=(t == 0), stop=(t == 2), perf_mode=DR)
                nc.vector.tensor_copy(Wt[:, cj, 0:BPC * H], pw[:])

            # ---- sT[n, (b h)] per batch: 15 DR matmuls over c ----
            ps_s = psS.tile([128, NT, BPC, H], F32, tag="S")
            for b in range(BPC):
                for nt in range(NT):
                    w = 128 if nt < NT - 1 else NTAIL
                    off = 128 * nt
                    for t in range(3):
                        nc.tensor.matmul(
                            ps_s[:w, nt, b, :],
                            xTs[b][:, 2 * t:2 * t + 2, off:off + w],
                            Wt[:, 2 * t:2 * t + 2, H * b:H * (b + 1)],
                            start=(t == 0), stop=(t == 2), perf_mode=DR)

            # ---- pT = exp(sT - 1), fp8 (the -1 cancels in 1/sum and
            #      keeps e below the fp8e4 max) ----
            nc.scalar.activation(
                pT[:, 0:4, :, 0:H], ps_s[:, 0:4, :, :], AF.Exp,
                bias=negone[:], scale=1.0)
            nc.scalar.activation(
                pT[:NTAIL, 4, :, 0:H], ps_s[:NTAIL, 4, :, :], AF.Exp,
                bias=negone[:NTAIL, :], scale=1.0)

            # ---- sums over n via ones-matmuls; rden = 1/sums ----
            pr = psR.tile([128, 192], F32, tag="R")
            for nt in range(NT):
                w = 128 if nt < NT - 1 else NTAIL
                nc.tensor.matmul(
                    pr[0:1, 0:96], ones8[:w, 0, 0:1],
                    pT[:w, nt, :, 0:H],
                    start=(nt == 0), stop=(nt == NT - 1))
            nc.vector.reciprocal(rden[:], pr[0:1, 0:96])

            # ---- rdenB[o, (b h)]: broadcast rden down 128 partitions with
            #      an outer-product matmul ----
            nc.tensor.matmul(
                pr[:, 96:192], onesf[:], rden[:], start=True, stop=True)
            nc.vector.tensor_copy(
                rdenB[:].rearrange("p b h -> p (b h)"), pr[:, 96:192])

            # ---- ZT[c, b-col] per batch: 18 DR matmuls + normalize-and-
            #      cast evacuation (runs as each x2 batch lands) ----
            po = psA.tile([128, CT, BPC], F32, tag="A")
            for b in range(BPC):
                pz = psZ.tile([128, CT, H], F32, tag="Z")
                x2 = x2s[b]
                for ci in range(CT):
                    for t in range(2):
                        nc.tensor.matmul(
                            pz[:, ci, :],
                            x2[:, 2 * t:2 * t + 2, 128 * ci:128 * (ci + 1)],
                            pT[:, 2 * t:2 * t + 2, b, 0:H],
                            start=(t == 0), stop=False, perf_mode=DR)
                    nc.tensor.matmul(
                        pz[:, ci, :],
                        x2ts[b][:, 128 * ci:128 * (ci + 1)],
                        pT[:NTAIL, 4, b, 0:H],
                        start=False, stop=True)
                nc.vector.tensor_mul(
                    ZT[:, :, b, 0:H], pz[:],
                    rdenB[:, b:b + 1, :].to_broadcast([128, CT, H]))

            # ---- oT per group (4/3/1 batches) so it tracks x2 arrivals.
            # non-DR: DoubleRow + dst partition 64 fails the walrus ISA
            # check (s3d3_mm_valid_dst_partition); cost is per-out-column
            # anyway so plain fp8 matmuls are the same speed here ----
            for js, jn in ((0, 4), (4, 3), (7, 1)):
                for ci in range(CT):
                    for hh in range(2):
                        h = 2 * ci + hh
                        base = 128 * ci + 64 * hh
                        for t in range(CT):
                            nc.tensor.matmul(
                                po[64 * hh:64 * (hh + 1), ci, js:js + jn],
                                wv[:, t, base:base + 64],
                                ZT[:, t, js:js + jn, h],
                                start=(t == 0), stop=(t == CT - 1),
                                tile_position=(0, 64 * hh))
                nc.vector.tensor_copy(
                    oT[:, :, js:js + jn], po[:, :, js:js + jn])

            # ---- clsT[j, b] = wp.T @ oT + pbT: the only work that waits
            #      for wp (the last DMA); one add, one output DMA ----
            pc = psA.tile([128, CT, BPC], F32, tag="A")
            for jc in range(CT):
                for i, t in enumerate((0, 1, 2)):
                    nc.tensor.matmul(
                        pc[:, jc, :],
                        wp[:, 2 * t:2 * t + 2, 128 * jc:128 * (jc + 1)],
                        oT[:, 2 * t:2 * t + 2, 0:BPC],
                        start=(i == 0), stop=(i == 2), perf_mode=DR)
            nc.vector.tensor_add(clsT_sb[:], pc[:], pbT[:])
            nc.sync.dma_start(clsT_d.ap(), clsT_sb[:])

    nc.compile()
    return nc


@functools.lru_cache(maxsize=1)
def _module():
    return build_module()


def make_in_maps(x, qkv_w, qkv_b, proj_w, proj_b):
    x = np.asarray(x, dtype=np.float32)
    qkv_w = np.asarray(qkv_w, dtype=np.float32)
    qkv_b = np.asarray(qkv_b, dtype=np.float32)
    proj_w = np.asarray(proj_w, dtype=np.float32)
    proj_b = np.asarray(proj_b, dtype=np.float32)

    wq = np.ascontiguousarray(qkv_w[:C].T * SCALE).astype(NPF8)     # [c, o]
    wk2 = np.ascontiguousarray(qkv_w[C:2 * C]).astype(NPF8)         # [o, c]
    wv = np.ascontiguousarray(qkv_w[2 * C:].T).astype(NPF8)         # [c, o]
    wp = np.ascontiguousarray(proj_w.T).astype(NPF8)                # [c, o]
    # q-bias folds into the Qblk blockdiag scatter: qbd[o] = qb[o]*SCALE
    qbd = np.ascontiguousarray(
        (qkv_b[:C] * SCALE).reshape(CT, 128).T).astype(np.float32)  # [p, a]
    # v bias contributes exactly (vb @ proj_w.T) to cls; fold into proj bias
    pb_eff = proj_b + qkv_b[2 * C:] @ proj_w.T

    in_maps = []
    for i in range(NCORES):
        xs = x[i * BPC:(i + 1) * BPC]                               # [8, N, C]
        x2 = xs.reshape(BPC * N, C).astype(NPF8)
        xT = np.ascontiguousarray(xs.transpose(2, 0, 1)).astype(NPF8)
        xcls = np.ascontiguousarray(
            xs[:, 0, :].T.reshape(CT, 128, BPC).transpose(1, 0, 2)
        ).astype(NPF8)                                              # [p, a, b]
        pbT = np.ascontiguousarray(
            np.tile(pb_eff[:, None], (1, BPC)).reshape(CT, 128, BPC)
            .transpose(1, 0, 2)).astype(np.float32)                 # [p, a, b]
        in_maps.append({
            "xT": xT, "x2": x2, "wq": wq, "wk2": wk2, "wv": wv, "wp": wp,
            "xcls": xcls, "qbd": qbd, "pbT": pbT,
        })
    return in_maps


def kernel(x, qkv_w, qkv_b, proj_w, proj_b):
    nc = _module()
    in_maps = make_in_maps(x, qkv_w, qkv_b, proj_w, proj_b)
    res = bass_utils.run_bass_kernel_spmd(
        nc, in_maps, core_ids=list(range(NCORES)))
    out = np.array(np.asarray(x), dtype=np.float32, copy=True)
    for i in range(NCORES):
        clsT = res.results[i]["clsT"].astype(np.float32)            # [p, a, b]
        out[i * BPC:(i + 1) * BPC, 0, :] = (
            clsT.transpose(2, 1, 0).reshape(BPC, C))
    return out


# revision 50
# speedup vs baseline: 1.0306x; 1.0306x over previous
"""ClassAttention kernel for 8x TRN2 NeuronCores — fp8 DoubleRow rewrite.

Reference computation (per batch element):
    qkv = x @ qkv_w.T + qkv_b                      # [N, 3C]
    q, k, v = split(qkv)                           # heads H=12, D=64
    s = softmax((q_cls . k) / sqrt(D))             # class-token query only
    cls = (s @ v) @ proj_w.T + proj_b              # [1, C]
    out = concat([cls, x[1:]])                     # rows 1..N pass through

Only the class token row changes, so the device computes just the [B, C]
cls output (shipped transposed as clsT in a descriptor-friendly
[128, 6, B] layout); rows 1..N pass through on the host.  Data-parallel
over batch: 8 batches per core, no collectives.

Algebraic structure (inherited from the bf16 baseline):
  - k-projection folds into x-space:  s[b,h,n] = sum_c Wt[c,bh] x[b,n,c]
    with Wt = wk.T @ blockdiag(q + qb) computed once on device; no k
    vector is materialized.  k-bias cancels in softmax; q-bias is added
    during the Qblk blockdiag scatter (per-partition scalar add).
  - v-projection commutes with the attention average: the kernel averages
    x (ZT = x.T @ p) and projects through wv once; v-bias folds into the
    proj bias on the host.
  - softmax skips the max-shift; exp(s - 1) keeps the fp8 range safe and
    the constant cancels in the 1/sum, which is applied per (b,h) column
    during the ZT psum evacuation.

What is new vs the 82.2us bf16 baseline (2.5x):
  - everything is fp8(e4m3) except psum (f32), the exp sums/rden (f32)
    and the cls output (bf16, widened to f32 on the host).  Error is dominated by the score-path
    quantization; measured full-output rel err 9.9e-3 vs the 2e-2 gate
    on the fixed-seed reference.  DoubleRow fp8 matmuls (two 128-row
    k-tiles per instruction, 0.5 cycles/row) carry all heavy
    contractions; fp8 operand tiles are padded so every DoubleRow k-pair
    slice has a 64-multiple byte stride (walrus ISA requirement), and
    DoubleRow outputs always sit at psum partition 0 (dst partition 64
    fails the s3d3 ISA check; those matmuls run non-DR).
  - every stage computes the TRANSPOSED output with a small moving free
    dim (qT, sT directly in [n, (b h)] form, ZT, oT head-diagonal blocks
    directly, clsT), so the kernel contains ZERO PE transposes of data
    and psum evacuations are few and wide.
  - 30 exact-size DMAs instead of 67 (HWDGE issue is ~630ns each and
    gated the baseline).  Modeled DMA stream is gapless at 360 GB/s:
    wq,wk2,xcls,qbd,wv | xT per batch | x2 per batch (512-row body +
    65-row tail) | wp, pbT last — so the only post-stream work is
    proj (18 DR matmuls) -> bias add -> one output DMA, and the last
    x2 batch's Z/oT chain hides under the wp transfer.
  - small tensors (xcls, qbd, pbT, clsT) use partition-major [128,6,k]
    DRAM layouts: 128 descriptors instead of 768 (descriptor-floor
    bound at 7ns/desc).

Per-core stages (b = 0..8 batches, c in 6 chunks of 128):
  qT[o, b]       36 matmuls          Qblk[o, (b h)]  12 scatter+qb adds
  Wt[c, (b h)]   18 DR + 6 copies    sT[n, (b h)]    120 DR matmuls
  pT = exp(sT-1) 2 Act ops -> fp8    sums/rden       5 ones-matmuls + recip
  rdenB[o,(b h)] 1 outer-product     ZT[c, b]        144 DR + 8 TT-mul evacs
  oT[o', b]      216 small matmuls (groups 4/3/1, tracks x2 arrivals)
  clsT[j, b]     18 DR + pbT add -> one output DMA

Modeled 32643 ns; measured full-output rel err 1.03e-2 (gate 2e-2).
"""

import functools

import numpy as np
import ml_dtypes

import concourse.bass as bass
import concourse.tile as tile
from concourse import bacc, mybir
from concourse import bass_utils

BF16 = mybir.dt.bfloat16
F8 = mybir.dt.float8e4
F32 = mybir.dt.float32
NPBF16 = ml_dtypes.bfloat16
NPF8 = ml_dtypes.float8_e4m3
DR = mybir.MatmulPerfMode.DoubleRow

B, N, C = 64, 577, 768
H, D = 12, 64
NCORES = 8
BPC = B // NCORES          # 8 batches per core
CT = C // 128              # 6 chunks of the feature dim
NT = 5                     # token tiles of 128 (last holds 65)
NTAIL = N - 4 * 128        # 65
SCALE = D ** -0.5          # folded into wq on the host


def build_module():
    nc = bacc.Bacc("TRN2", target_bir_lowering=False, debug=False)

    xT_d = nc.dram_tensor("xT", [C, BPC, N], F8, kind="ExternalInput")
    x2_d = nc.dram_tensor("x2", [BPC * N, C], F8, kind="ExternalInput")
    wq_d = nc.dram_tensor("wq", [C, C], F8, kind="ExternalInput")      # [c, o]
    wk2_d = nc.dram_tensor("wk2", [C, C], F8, kind="ExternalInput")    # [o, c]
    wv_d = nc.dram_tensor("wv", [C, C], F8, kind="ExternalInput")      # [c, o]
    wp_d = nc.dram_tensor("wp", [C, C], F8, kind="ExternalInput")      # [c, o]
    xcls_d = nc.dram_tensor("xcls", [128, CT, BPC], F8, kind="ExternalInput")
    qbd_d = nc.dram_tensor("qbd", [128, CT], F32, kind="ExternalInput")
    pbT_d = nc.dram_tensor("pbT", [128, CT, BPC], F32, kind="ExternalInput")
    clsT_d = nc.dram_tensor("clsT", [128, CT, BPC], BF16, kind="ExternalOutput")

    AF = mybir.ActivationFunctionType

    with tile.TileContext(nc) as tc:
        with (
            tc.tile_pool(name="sb", bufs=1) as sb,
            tc.tile_pool(name="psA", bufs=2, space="PSUM") as psA,
            tc.tile_pool(name="psW", bufs=1, space="PSUM") as psW,
            tc.tile_pool(name="psS", bufs=1, space="PSUM") as psS,
            tc.tile_pool(name="psR", bufs=1, space="PSUM") as psR,
            tc.tile_pool(name="psZ", bufs=3, space="PSUM") as psZ,
        ):
            # ---- DMAs, in consumption order (one channel, serialized) ----
            wq = sb.tile([128, CT, C], F8, tag="wq")
            nc.sync.dma_start(
                wq[:], wq_d.ap().rearrange("(a p) o -> p a o", p=128))
            wk2 = sb.tile([128, CT, C], F8, tag="wk2")
            nc.sync.dma_start(
                wk2[:], wk2_d.ap().rearrange("(a p) o -> p a o", p=128))
            xcls = sb.tile([128, CT, BPC], F8, tag="xcls")
            nc.sync.dma_start(xcls[:], xcls_d.ap())
            qbd = sb.tile([128, CT, 1], F32, tag="qbd")
            nc.sync.dma_start(qbd[:, :, 0], qbd_d.ap())
            # x in c-major layout, one DMA per batch; rows padded to 640 so
            # DoubleRow k-tile-pair slices have a 64-multiple stride (walrus
            # ISA requirement on Ldweights)
            wv = sb.tile([128, CT, C], F8, tag="wv")
            nc.sync.dma_start(
                wv[:], wv_d.ap().rearrange("(a p) o -> p a o", p=128))
            xTs = []
            for b in range(BPC):
                xt = sb.tile([128, CT, 640], F8, tag=f"xT{b}")
                nc.sync.dma_start(
                    xt[:, :, 0:N],
                    xT_d.ap()[:, b, :].rearrange("(a p) t -> p a t", p=128))
                xTs.append(xt)
            # x in token-major layout, two exact-size DMAs per batch (the
            # 512-row body, then the 65-row tail)
            x2s = []
            x2ts = []
            for b in range(BPC):
                x2 = sb.tile([128, 4, C], F8, tag=f"x2{b}")
                nc.sync.dma_start(
                    x2[:],
                    x2_d.ap()[b * N:b * N + 512, :]
                    .rearrange("(a p) c -> p a c", p=128))
                x2t = sb.tile([NTAIL, C], F8, tag=f"x2t{b}")
                nc.sync.dma_start(
                    x2t[:], x2_d.ap()[b * N + 512:b * N + N, :])
                x2s.append(x2)
                x2ts.append(x2t)
            # wp is the LAST input: everything up to oT overlaps the input
            # stream, so the only post-stream work is proj -> add -> out DMA
            wp = sb.tile([128, CT, C], F8, tag="wp")
            nc.sync.dma_start(
                wp[:], wp_d.ap().rearrange("(a p) o -> p a o", p=128))
            # pbT is the very last input: the only work behind it is the
            # final bias add
            pbT = sb.tile([128, CT, BPC], F32, tag="pbT")
            nc.sync.dma_start(pbT[:], pbT_d.ap())

            # ---- small constants ----
            ones8 = sb.tile([128, 2, 64], F8, tag="ones8")
            nc.vector.memset(ones8[:], 1.0)
            negone = sb.tile([128, 1], F32, tag="negone")
            nc.vector.memset(negone[:], -1.0)
            onesf = sb.tile([1, 128], F32, tag="onesf")
            nc.vector.memset(onesf[:], 1.0)
            Qblk = sb.tile([128, CT, 128], F8, tag="Qblk")
            nc.vector.memset(Qblk[:], 0.0)

            # fp8 operand tiles are padded so every DoubleRow k-pair slice
            # has a 64-multiple stride
            Wt = sb.tile([128, CT, 128], F8, tag="Wt")
            pT = sb.tile([128, NT, BPC, 16], F8, tag="pT")
            rden = sb.tile([1, BPC * H], F32, tag="rden")
            rdenB = sb.tile([128, BPC, H], F32, tag="rdenB")
            ZT = sb.tile([128, CT, BPC, 16], F8, tag="ZT")
            oT = sb.tile([128, CT, 64], F8, tag="oT")
            clsT_sb = sb.tile([128, CT, BPC], BF16, tag="clsT_sb")

            # ---- qT[o, b]: 36 bf16 matmuls, out free dim 8 ----
            pq = psA.tile([128, CT, BPC], F32, tag="A")
            for oc in range(CT):
                for ck in range(CT):
                    nc.tensor.matmul(
                        pq[:, oc, :],
                        wq[:, ck, 128 * oc:128 * (oc + 1)],
                        xcls[:, ck, :],
                        start=(ck == 0), stop=(ck == CT - 1))

            # ---- Qblk[o, (b h)]: blockdiag scatter of qT + q-bias
            #      (per-partition scalar add), cast fp8 ----
            QblkV = Qblk[:, :, 0:BPC * H].rearrange(
                "p a (b h) -> p a b h", h=H)
            for oc in range(CT):
                for j in range(2):
                    h = 2 * oc + j
                    nc.vector.tensor_scalar_add(
                        QblkV[64 * j:64 * (j + 1), oc, :, h],
                        pq[64 * j:64 * (j + 1), oc, :],
                        qbd[64 * j:64 * (j + 1), oc, :])

            # ---- Wt[c, (b h)] = wk2.T @ Qblk + wtqb, cast fp8 ----
            for cj in range(CT):
                pw = psW.tile([128, BPC * H], F32, tag="W")
                for t in range(3):
                    nc.tensor.matmul(
                        pw[:], wk2[:, 2 * t:2 * t + 2, 128 * cj:128 * (cj + 1)],
                        Qblk[:, 2 * t:2 * t + 2, 0:BPC * H],
                        start=(t == 0), stop=(t == 2), perf_mode=DR)
                nc.vector.tensor_copy(Wt[:, cj, 0:BPC * H], pw[:])

            # ---- sT[n, (b h)] per batch: 15 DR matmuls over c ----
            ps_s = psS.tile([128, NT, BPC, H], F32, tag="S")
            for b in range(BPC):
                for nt in range(NT):
                    w = 128 if nt < NT - 1 else NTAIL
                    off = 128 * nt
                    for t in range(3):
                        nc.tensor.matmul(
                            ps_s[:w, nt, b, :],
                            xTs[b][:, 2 * t:2 * t + 2, off:off + w],
                            Wt[:, 2 * t:2 * t + 2, H * b:H * (b + 1)],
                            start=(t == 0), stop=(t == 2), perf_mode=DR)

            # ---- pT = exp(sT - 1), fp8 (the -1 cancels in 1/sum and
            #      keeps e below the fp8e4 max) ----
            nc.scalar.activation(
                pT[:, 0:4, :, 0:H], ps_s[:, 0:4, :, :], AF.Exp,
                bias=negone[:], scale=1.0)
            nc.scalar.activation(
                pT[:NTAIL, 4, :, 0:H], ps_s[:NTAIL, 4, :, :], AF.Exp,
                bias=negone[:NTAIL, :], scale=1.0)

            # ---- sums over n via ones-matmuls; rden = 1/sums ----
            pr = psR.tile([128, 192], F32, tag="R")
            for nt in range(NT):
                w = 128 if nt < NT - 1 else NTAIL
                nc.tensor.matmul(
                    pr[0:1, 0:96], ones8[:w, 0, 0:1],
                    pT[:w, nt, :, 0:H],
                    start=(nt == 0), stop=(nt == NT - 1))
            nc.vector.reciprocal(rden[:], pr[0:1, 0:96])

            # ---- rdenB[o, (b h)]: broadcast rden down 128 partitions with
            #      an outer-product matmul ----
            nc.tensor.matmul(
                pr[:, 96:192], onesf[:], rden[:], start=True, stop=True)
            nc.vector.tensor_copy(
                rdenB[:].rearrange("p b h -> p (b h)"), pr[:, 96:192])

            # ---- ZT[c, b-col] per batch: 18 DR matmuls + normalize-and-
            #      cast evacuation (runs as each x2 batch lands) ----
            po = psA.tile([128, CT, BPC], F32, tag="A")
            for b in range(BPC):
                pz = psZ.tile([128, CT, H], F32, tag="Z")
                x2 = x2s[b]
                for ci in range(CT):
                    for t in range(2):
                        nc.tensor.matmul(
                            pz[:, ci, :],
                            x2[:, 2 * t:2 * t + 2, 128 * ci:128 * (ci + 1)],
                            pT[:, 2 * t:2 * t + 2, b, 0:H],
                            start=(t == 0), stop=False, perf_mode=DR)
                    nc.tensor.matmul(
                        pz[:, ci, :],
                        x2ts[b][:, 128 * ci:128 * (ci + 1)],
                        pT[:NTAIL, 4, b, 0:H],
                        start=False, stop=True)
                nc.vector.tensor_mul(
                    ZT[:, :, b, 0:H], pz[:],
                    rdenB[:, b:b + 1, :].to_broadcast([128, CT, H]))

            # ---- oT per group (4/3/1 batches) so it tracks x2 arrivals.
            # non-DR: DoubleRow + dst partition 64 fails the walrus ISA
            # check (s3d3_mm_valid_dst_partition); cost is per-out-column
            # anyway so plain fp8 matmuls are the same speed here ----
            for js, jn in ((0, 4), (4, 3), (7, 1)):
                for ci in range(CT):
                    for hh in range(2):
                        h = 2 * ci + hh
                        base = 128 * ci + 64 * hh
                        for t in range(CT):
                            nc.tensor.matmul(
                                po[64 * hh:64 * (hh + 1), ci, js:js + jn],
                                wv[:, t, base:base + 64],
                                ZT[:, t, js:js + jn, h],
                                start=(t == 0), stop=(t == CT - 1),
                                tile_position=(0, 64 * hh))
                nc.vector.tensor_copy(
                    oT[:, :, js:js + jn], po[:, :, js:js + jn])

            # ---- clsT[j, b] = wp.T @ oT + pbT: the only work that waits
            #      for wp (the last DMA); one add, one output DMA ----
            pc = psA.tile([128, CT, BPC], F32, tag="A")
            for jc in range(CT):
                for t in range(3):
                    nc.tensor.matmul(
                        pc[:, jc, :],
                        wp[:, 2 * t:2 * t + 2, 128 * jc:128 * (jc + 1)],
                        oT[:, 2 * t:2 * t + 2, 0:BPC],
                        start=(t == 0), stop=(t == 2), perf_mode=DR)
            nc.vector.tensor_add(clsT_sb[:], pc[:], pbT[:])
            nc.sync.dma_start(clsT_d.ap(), clsT_sb[:])

    nc.compile()
    return nc


@functools.lru_cache(maxsize=1)
def _module():
    return build_module()


def make_in_maps(x, qkv_w, qkv_b, proj_w, proj_b):
    x = np.asarray(x, dtype=np.float32)
    qkv_w = np.asarray(qkv_w, dtype=np.float32)
    qkv_b = np.asarray(qkv_b, dtype=np.float32)
    proj_w = np.asarray(proj_w, dtype=np.float32)
    proj_b = np.asarray(proj_b, dtype=np.float32)

    wq = np.ascontiguousarray(qkv_w[:C].T * SCALE).astype(NPF8)     # [c, o]
    wk2 = np.ascontiguousarray(qkv_w[C:2 * C]).astype(NPF8)         # [o, c]
    wv = np.ascontiguousarray(qkv_w[2 * C:].T).astype(NPF8)         # [c, o]
    wp = np.ascontiguousarray(proj_w.T).astype(NPF8)                # [c, o]
    # q-bias folds into the Qblk blockdiag scatter: qbd[o] = qb[o]*SCALE
    qbd = np.ascontiguousarray(
        (qkv_b[:C] * SCALE).reshape(CT, 128).T).astype(np.float32)  # [p, a]
    # v bias contributes exactly (vb @ proj_w.T) to cls; fold into proj bias
    pb_eff = proj_b + qkv_b[2 * C:] @ proj_w.T

    in_maps = []
    for i in range(NCORES):
        xs = x[i * BPC:(i + 1) * BPC]                               # [8, N, C]
        x2 = xs.reshape(BPC * N, C).astype(NPF8)
        xT = np.ascontiguousarray(xs.transpose(2, 0, 1)).astype(NPF8)
        xcls = np.ascontiguousarray(
            xs[:, 0, :].T.reshape(CT, 128, BPC).transpose(1, 0, 2)
        ).astype(NPF8)                                              # [p, a, b]
        pbT = np.ascontiguousarray(
            np.tile(pb_eff[:, None], (1, BPC)).reshape(CT, 128, BPC)
            .transpose(1, 0, 2)).astype(np.float32)                 # [p, a, b]
        in_maps.append({
            "xT": xT, "x2": x2, "wq": wq, "wk2": wk2, "wv": wv, "wp": wp,
            "xcls": xcls, "qbd": qbd, "pbT": pbT,
        })
    return in_maps


def kernel(x, qkv_w, qkv_b, proj_w, proj_b):
    nc = _module()
    in_maps = make_in_maps(x, qkv_w, qkv_b, proj_w, proj_b)
    res = bass_utils.run_bass_kernel_spmd(
        nc, in_maps, core_ids=list(range(NCORES)))
    out = np.array(np.asarray(x), dtype=np.float32, copy=True)
    for i in range(NCORES):
        clsT = res.results[i]["clsT"].astype(np.float32)            # [p, a, b]
        out[i * BPC:(i + 1) * BPC, 0, :] = (
            clsT.transpose(2, 1, 0).reshape(BPC, C))
    return out
